# revision 10
# baseline (speedup 1.0000x reference)
#!/usr/bin/env python3
"""2-layer GAT on 8 NeuronCores (Bass/Tile).

Sharding: nodes partitioned across 8 cores by dst id (graph parallel).
Per-node features computed locally, per-node gather tables allgathered,
per-edge source rows fetched with dma_gather, segment softmax/aggregation
via indicator matmuls on the tensor engine.
"""
import sys
import numpy as np

sys.path.insert(0, "/opt/pypackages")
sys.path.insert(0, "/opt/trn_rl_repo")

import concourse.bass as bass
import concourse.bacc as bacc
import concourse.tile as tile
import concourse.mybir as mybir
from concourse.bass_utils import run_bass_kernel_spmd

# problem constants
N = 100000
F_IN = 512
NHID = 16
HEADS = 8
NCLASS = 40
E = 1600000
NEG_SLOPE = 0.2

NCORES = 8
NPC = N // NCORES            # 12500 nodes per core
DCH = 128                    # dsts per chunk
NCH = (NPC + DCH - 1) // DCH  # 98 chunks
NPAD = NCH * DCH             # 12544 padded rows per core shard
NSCH = 4
SCHW = (NPAD * NCORES) // NSCH  # 25088 src rows per index window (int16-safe)

ROW1 = 256    # fp16 elems per L1 table row (512B): [h1 128 | asrc1 8 | pad]
ROW2 = 128    # fp16 elems per L2 table row (256B): [h2 40 | one | asrc2 | pad]
ROWA = 128    # fp16 elems per adst-replica row (256B)

F16 = mybir.dt.float16
F32 = mybir.dt.float32
I16 = mybir.dt.int16


def _wrap_block(v):
    """Wrap a 1-D int16 block (len % 16 == 0) into dma_gather idx layout
    [16, L/16], replicated to 128 partitions."""
    w = v.reshape(-1, 16).T
    return np.tile(w, (8, 1))


def _prep(x, edge_index, W1, att_src1, att_dst1, W2, att_src2, att_dst2):
    """Host-side sharding/packing. Returns (in_maps, shapes) where shapes is
    the static cell structure shared by all cores."""
    src = np.concatenate([edge_index[0], np.arange(N, dtype=np.int64)])
    dst = np.concatenate([edge_index[1], np.arange(N, dtype=np.int64)])

    core = dst // NPC
    dl = (dst - core * NPC).astype(np.int64)      # local dst 0..12499
    dch = dl >> 7                                  # dst chunk 0..97
    s_pad = (src // NPC) * NPAD + (src % NPC)      # padded global src row
    sch = s_pad // SCHW
    sloc = (s_pad - sch * SCHW).astype(np.int64)   # 0..25087 (int16 ok)

    cell = ((core * NCH + dch) * NSCH + sch).astype(np.int64)
    order = np.argsort(cell * (SCHW + 1) + sloc, kind="stable")
    cell_s, sloc_s, dl_s = cell[order], sloc[order], dl[order]

    ncells = NCORES * NCH * NSCH
    counts = np.bincount(cell_s, minlength=ncells).reshape(NCORES, NCH * NSCH)
    shapes = (np.ceil(counts.max(axis=0) / 128.0).astype(np.int64) * 128)  # [NCH*NSCH]
    cell_starts = np.concatenate([[0], np.cumsum(shapes)])                 # per-core stream offsets
    t_total = int(cell_starts[-1]) // 128

    # rank of each edge within its cell
    group_start = np.concatenate([[0], np.cumsum(counts.reshape(-1))])
    first_of_cell = group_start[cell_s]
    rank = np.arange(len(cell_s)) - first_of_cell
    # destination position within the owning core's padded stream
    pos = cell_starts[cell_s % (NCH * NSCH)] + rank
    core_s = cell_s // (NCH * NSCH)

    L = t_total * 128
    idx1 = np.zeros((NCORES, L), dtype=np.int16)
    idxd = np.zeros((NCORES, L), dtype=np.int16)
    dstloc = np.full((NCORES, L), 255.0, dtype=np.float16)
    idx1[core_s, pos] = sloc_s.astype(np.int16)
    idxd[core_s, pos] = dl_s.astype(np.int16)
    dstloc[core_s, pos] = (dl_s & 127).astype(np.float16)

    # per-chunk tile counts and cell layout
    shapes2 = shapes.reshape(NCH, NSCH)
    # wrapped idx streams
    IDX1 = np.zeros((NCORES, 128, L // 16), dtype=np.int16)
    IDXD = np.zeros((NCORES, 128, L // 16), dtype=np.int16)
    for k in range(NCORES):
        off = 0
        for d in range(NCH):
            chunk_len = int(shapes2[d].sum())
            if chunk_len:
                blk = idxd[k, off:off + chunk_len]
                IDXD[k][:, off // 16:(off + chunk_len) // 16] = _wrap_block(blk)
            coff = off
            for s in range(NSCH):
                cl = int(shapes2[d, s])
                if cl:
                    blk = idx1[k, coff:coff + cl]
                    IDX1[k][:, coff // 16:(coff + cl) // 16] = _wrap_block(blk)
                coff += cl
            off += chunk_len
    DSTLOC = dstloc.reshape(NCORES, t_total, 128).transpose(0, 2, 1).copy()

    # weights
    asrc1 = att_src1.reshape(HEADS, NHID)
    adst1 = att_dst1.reshape(HEADS, NHID)
    W1r = W1.reshape(F_IN, HEADS, NHID)
    W1as = np.einsum("khc,hc->kh", W1r, asrc1)     # [512, 8]
    W1ad = np.einsum("khc,hc->kh", W1r, adst1)
    W1ext = np.concatenate([W1, W1as, W1ad], axis=1).astype(np.float16)  # [512, 144]
    W2as = W2 @ att_src2.reshape(NCLASS, 1)        # [128, 1]
    W2ad = W2 @ att_dst2.reshape(NCLASS, 1)
    W2ext = np.concatenate([W2, W2as, W2ad], axis=1).astype(np.float16)  # [128, 42]

    iota = np.broadcast_to(np.arange(128, dtype=np.float16), (128, 128)).copy()

    in_maps = []
    for k in range(NCORES):
        xs = x[k * NPC:(k + 1) * NPC]              # [12500, 512]
        xT = np.zeros((F_IN, NPAD), dtype=np.float16)
        xT[:, :NPC] = xs.T
        in_maps.append({
            "xT": xT,
            "W1ext": W1ext,
            "W2ext": W2ext,
            "IDX1": IDX1[k],
            "IDXD": IDXD[k],
            "DSTLOC": DSTLOC[k],
            "iota": iota,
        })
    return in_maps, shapes2


def _build(shapes2, nch=NCH, phases="ABCDE", clevel=9):
    """Build the Bass module given the static cell structure [NCH, NSCH]."""
    from concourse.masks import make_identity

    t_chunks = [int(shapes2[d].sum()) // 128 for d in range(NCH)]
    t_total = sum(t_chunks)
    TMAX = max(t_chunks)

    nc = bacc.Bacc("TRN2", target_bir_lowering=False, debug=False,
                   enable_asserts=False, num_devices=NCORES)

    xT = nc.dram_tensor("xT", [F_IN, NPAD], F16, kind="ExternalInput")
    W1e = nc.dram_tensor("W1ext", [F_IN, 144], F16, kind="ExternalInput")
    W2e = nc.dram_tensor("W2ext", [128, 42], F16, kind="ExternalInput")
    IDX1 = nc.dram_tensor("IDX1", [128, t_total * 8], I16, kind="ExternalInput")
    IDXD = nc.dram_tensor("IDXD", [128, t_total * 8], I16, kind="ExternalInput")
    DSTLOC = nc.dram_tensor("DSTLOC", [128, t_total], F16, kind="ExternalInput")
    IOTA = nc.dram_tensor("iota", [128, 128], F16, kind="ExternalInput")
    OUT = nc.dram_tensor("out", [NPAD, NCLASS], F32, kind="ExternalOutput")

    tab1_sh = nc.dram_tensor("tab1_sh", [NPAD, ROW1], F16, kind="Internal")
    tab1 = nc.dram_tensor("tab1", [NPAD * NCORES, ROW1], F16, kind="Internal",
                          addr_space="Shared")
    tab2_sh = nc.dram_tensor("tab2_sh", [NPAD, ROW2], F16, kind="Internal")
    tab2 = nc.dram_tensor("tab2", [NPAD * NCORES, ROW2], F16, kind="Internal",
                          addr_space="Shared")
    adr1 = nc.dram_tensor("adr1", [NPAD, ROWA], F16, kind="Internal")
    adr2 = nc.dram_tensor("adr2", [NPAD, ROWA], F16, kind="Internal")

    eq = mybir.AluOpType.is_equal
    mult = mybir.AluOpType.mult
    amax = mybir.AluOpType.max
    aadd = mybir.AluOpType.add
    sub = mybir.AluOpType.subtract
    AF = mybir.ActivationFunctionType
    AX = mybir.AxisListType

    with tile.TileContext(nc) as tc:
        if "A" in phases:
            _phase_a(nc, tc, nch, xT, W1e, tab1_sh, adr1)
        if "B" in phases:
            tc.strict_bb_all_engine_barrier()
            nc.gpsimd.collective_compute(
                "AllGather", mybir.AluOpType.bypass,
                replica_groups=[list(range(NCORES))],
                ins=[tab1_sh[:]], outs=[tab1[:]])
            tc.strict_bb_all_engine_barrier()
        if "C" in phases:
            _phase_c(nc, tc, nch, shapes2, t_chunks, TMAX, make_identity,
                     IDX1, IDXD, DSTLOC, IOTA, W2e, tab1, adr1, tab2_sh, adr2,
                     eq, mult, amax, aadd, AF, clevel)
        if "D" in phases:
            tc.strict_bb_all_engine_barrier()
            nc.gpsimd.collective_compute(
                "AllGather", mybir.AluOpType.bypass,
                replica_groups=[list(range(NCORES))],
                ins=[tab2_sh[:]], outs=[tab2[:]])
            tc.strict_bb_all_engine_barrier()
        if "E" in phases:
            _phase_e(nc, tc, nch, shapes2, t_chunks, TMAX,
                     IDX1, IDXD, DSTLOC, IOTA, tab2, adr2, OUT,
                     eq, mult, amax, aadd, sub, AF, AX)

    nc.compile()
    return nc


def _phase_a(nc, tc, nch, xT, W1e, tab1_sh, adr1):
    with tc.tile_pool(name="sbA", bufs=1) as sba, \
         tc.tile_pool(name="sbA2", bufs=2) as sba2, \
         tc.tile_pool(name="psA", bufs=2, space="PSUM") as psa:
        xts = [sba.tile([128, NPAD], F16, tag=f"xt{k}", name=f"xt{k}")
               for k in range(4)]
        w1s = [sba.tile([128, 144], F16, tag=f"w1{k}", name=f"w1{k}")
               for k in range(4)]
        for k in range(4):
            nc.sync.dma_start(xts[k][:], xT[k * 128:(k + 1) * 128, :])
            nc.sync.dma_start(w1s[k][:], W1e[k * 128:(k + 1) * 128, :])
        for nt in range(nch):
            ps = psa.tile([128, 144], F32, tag="psA", name="psA")
            for k in range(4):
                nc.tensor.matmul(ps[:], lhsT=xts[k][:, nt * 128:(nt + 1) * 128],
                                 rhs=w1s[k][:], start=(k == 0), stop=(k == 3))
            row = sba2.tile([128, 136], F16, tag="row", name="row")
            nc.vector.tensor_copy(row[:], ps[:, 0:136])
            nc.sync.dma_start(tab1_sh[nt * 128:(nt + 1) * 128, 0:136], row[:])
            t8 = sba2.tile([128, 8], F16, tag="t8", name="t8")
            nc.vector.tensor_copy(t8[:], ps[:, 136:144])
            ar = sba2.tile([128, ROWA], F16, tag="ar", name="ar")
            nc.vector.tensor_copy(
                ar[:].rearrange("p (r h) -> p r h", h=8),
                t8[:].rearrange("p (r h) -> p r h", r=1).to_broadcast([128, 16, 8]))
            nc.sync.dma_start(adr1[nt * 128:(nt + 1) * 128, :], ar[:])


def _phase_c(nc, tc, nch, shapes2, t_chunks, TMAX, make_identity,
             IDX1, IDXD, DSTLOC, IOTA, W2e, tab1, adr1, tab2_sh, adr2,
             eq, mult, amax, aadd, AF, clevel=9):
    with tc.tile_pool(name="sbC", bufs=1) as sbc, \
         tc.tile_pool(name="sbC2", bufs=2) as sb2, \
         tc.tile_pool(name="psC", bufs=2, space="PSUM") as psc:
        iot = sbc.tile([128, 128], F16, tag="iota", name="iotc")
        nc.sync.dma_start(iot[:], IOTA[:])
        ident = sbc.tile([128, 128], F16, tag="ident", name="ident")
        make_identity(nc, ident[:])
        w2s = sbc.tile([128, 42], F16, tag="w2s", name="w2s")
        nc.sync.dma_start(w2s[:], W2e[:])

        off = 0  # tile offset into the edge stream
        for d in range(nch):
            T = t_chunks[d]
            if T == 0:
                continue
            i1 = sb2.tile([128, TMAX * 8], I16, tag="i1", name="i1")
            nc.sync.dma_start(i1[:, 0:T * 8], IDX1[:, off * 8:(off + T) * 8])
            idd = sb2.tile([128, TMAX * 8], I16, tag="idd", name="idd")
            nc.sync.dma_start(idd[:, 0:T * 8], IDXD[:, off * 8:(off + T) * 8])
            dlc = sb2.tile([128, TMAX], F16, tag="dlc", name="dlc")
            nc.sync.dma_start(dlc[:, 0:T], DSTLOC[:, off:off + T])

            g1 = sb2.tile([128, TMAX * ROW1], F16, tag="g1", name="g1")
            coff = 0
            for s in range(NSCH):
                cl = int(shapes2[d, s])
                if cl == 0:
                    continue
                if clevel >= 1:
                    nc.gpsimd.dma_gather(
                        out_ap=g1[:, coff * 2:(coff * 2 + (cl // 128) * ROW1)]
                        .rearrange("p (t e) -> p t e", e=ROW1),
                        in_ap=tab1[s * SCHW:(s + 1) * SCHW, :],
                        idxs_ap=i1[:, coff // 16:(coff + cl) // 16],
                        num_idxs=cl, num_idxs_reg=cl, elem_size=ROW1, single_packet=False)
                coff += cl
            ga = sb2.tile([128, TMAX * ROWA], F16, tag="ga", name="ga")
            nedge = T * 128
            nc.gpsimd.dma_gather(
                out_ap=ga[:, 0:T * ROWA].rearrange("p (t e) -> p t e", e=ROWA),
                in_ap=adr1[:],
                idxs_ap=idd[:, 0:nedge // 16],
                num_idxs=nedge, num_idxs_reg=nedge, elem_size=ROWA, single_packet=False)

            if clevel < 2:
                dbg = sb2.tile([128, 128], F16, tag="dbg", name="dbg")
                nc.vector.tensor_copy(dbg[:], ga[:, 0:128] if clevel < 1 else g1[:, 0:128])
                nc.sync.dma_start(tab2_sh[d * 128:(d + 1) * 128, 0:128], dbg[:])
                off += T
                continue
            g13 = g1[:, 0:T * ROW1].rearrange("p (t e) -> p t e", e=ROW1)
            ga3 = ga[:, 0:T * ROWA].rearrange("p (t e) -> p t e", e=ROWA)

            ind = sb2.tile([128, TMAX * 128], F16, tag="ind", name="ind")
            ind3 = ind[:, 0:T * 128].rearrange("p (t s) -> p t s", s=128)
            nc.vector.tensor_tensor(
                out=ind3,
                in0=iot[:].rearrange("p (t s) -> p t s", t=1)
                .to_broadcast([128, T, 128]),
                in1=dlc[:, 0:T].rearrange("p (t s) -> p t s", s=1)
                .to_broadcast([128, T, 128]),
                op=eq)

            att = sb2.tile([128, TMAX * 8], F16, tag="att", name="att")
            at3 = att[:, 0:T * 8].rearrange("p (t h) -> p t h", h=8)
            nc.vector.tensor_tensor(out=at3, in0=g13[:, :, 128:136],
                                    in1=ga3[:, :, 0:8], op=aadd)
            nc.vector.scalar_tensor_tensor(
                out=at3, in0=at3, scalar=NEG_SLOPE, in1=at3, op0=mult, op1=amax)
            wst = sb2.tile([128, TMAX * 8], F16, tag="wst", name="wst")
            nc.scalar.activation(out=wst[:, 0:T * 8], in_=att[:, 0:T * 8],
                                 func=AF.Exp)

            if clevel < 3:
                dbg = sb2.tile([128, 128], F16, tag="dbg", name="dbg")
                nc.vector.tensor_copy(dbg[:, 0:120], ind[:, 0:120])
                nc.vector.tensor_copy(dbg[:, 120:128], wst[:, 0:8])
                nc.sync.dma_start(tab2_sh[d * 128:(d + 1) * 128, 0:128], dbg[:])
                off += T
                continue
            ust = sb2.tile([128, TMAX * 136], F16, tag="ust", name="ust")
            us3 = ust[:, 0:T * 136].rearrange("p (t e) -> p t e", e=136)
            w3 = wst[:, 0:T * 8].rearrange("p (t h) -> p t h", h=8)
            for t in range(T):
                nc.vector.tensor_tensor(
                    out=us3[:, t, 0:128].rearrange("p (h c) -> p h c", c=NHID),
                    in0=g13[:, t, 0:128].rearrange("p (h c) -> p h c", c=NHID),
                    in1=w3[:, t, :].rearrange("p (h c) -> p h c", c=1)
                    .to_broadcast([128, 8, NHID]),
                    op=mult)
            nc.vector.tensor_copy(us3[:, :, 128:136], w3)

            ps1 = psc.tile([128, 136], F32, tag="ps1", name="ps1")
            for t in range(T):
                nc.tensor.matmul(ps1[:], lhsT=ind[:, t * 128:(t + 1) * 128],
                                 rhs=ust[:, t * 136:(t + 1) * 136],
                                 start=(t == 0), stop=(t == T - 1))

            if clevel < 4:
                dbg = sb2.tile([128, 128], F16, tag="dbg", name="dbg")
                nc.vector.tensor_copy(dbg[:], ps1[:, 0:128])
                nc.sync.dma_start(tab2_sh[d * 128:(d + 1) * 128, 0:128], dbg[:])
                off += T
                continue
            rc = sb2.tile([128, 8], F32, tag="rc", name="rc")
            nc.vector.reciprocal(rc[:], ps1[:, 128:136])
            o1 = sb2.tile([128, 128], F32, tag="o1", name="o1")
            nc.vector.tensor_tensor(
                out=o1[:].rearrange("p (h c) -> p h c", c=NHID),
                in0=ps1[:, 0:128].rearrange("p (h c) -> p h c", c=NHID),
                in1=rc[:].rearrange("p (h c) -> p h c", c=1)
                .to_broadcast([128, 8, NHID]),
                op=mult)
            # elu = max(x,0) + (exp(min(x,0)) - 1)
            t1 = sb2.tile([128, 128], F32, tag="t1", name="t1")
            nc.vector.tensor_scalar_min(t1[:], o1[:], 0.0)
            t2 = sb2.tile([128, 128], F32, tag="t2", name="t2")
            nc.scalar.activation(out=t2[:], in_=t1[:], func=AF.Exp)
            nc.vector.tensor_scalar_add(t2[:], t2[:], -1.0)
            nc.vector.tensor_scalar_max(t1[:], o1[:], 0.0)
            elu = sb2.tile([128, 128], F16, tag="elu", name="elu")
            nc.vector.tensor_tensor(out=elu[:], in0=t1[:], in1=t2[:], op=aadd)

            if clevel < 5:
                nc.sync.dma_start(tab2_sh[d * 128:(d + 1) * 128, 0:128], elu[:])
                off += T
                continue
            psT = psc.tile([128, 128], F16, tag="psT", name="psT")
            nc.tensor.transpose(psT[:], elu[:], ident[:])
            eluT = sb2.tile([128, 128], F16, tag="eluT", name="eluT")
            nc.vector.tensor_copy(eluT[:], psT[:])
            ps2a = psc.tile([128, 42], F32, tag="ps2a", name="ps2a")
            nc.tensor.matmul(ps2a[:], lhsT=eluT[:], rhs=w2s[:],
                             start=True, stop=True)

            h2r = sb2.tile([128, ROW2], F16, tag="h2r", name="h2r")
            nc.vector.tensor_copy(h2r[:, 0:NCLASS], ps2a[:, 0:NCLASS])
            nc.vector.memset(h2r[:, NCLASS:NCLASS + 1], 1.0)
            nc.vector.tensor_copy(h2r[:, NCLASS + 1:NCLASS + 2],
                                  ps2a[:, NCLASS:NCLASS + 1])
            nc.sync.dma_start(tab2_sh[d * 128:(d + 1) * 128, 0:NCLASS + 2],
                              h2r[:, 0:NCLASS + 2])
            a2c = sb2.tile([128, 1], F16, tag="a2c", name="a2c")
            nc.vector.tensor_copy(a2c[:], ps2a[:, 41:42])
            ar2 = sb2.tile([128, ROWA], F16, tag="ar2", name="ar2")
            nc.vector.tensor_copy(
                ar2[:].rearrange("p (r h) -> p r h", h=1),
                a2c[:].rearrange("p (r h) -> p r h", r=1)
                .to_broadcast([128, 128, 1]))
            nc.sync.dma_start(adr2[d * 128:(d + 1) * 128, :], ar2[:])
            off += T


def _phase_e(nc, tc, nch, shapes2, t_chunks, TMAX,
             IDX1, IDXD, DSTLOC, IOTA, tab2, adr2, OUT,
             eq, mult, amax, aadd, sub, AF, AX):
    with tc.tile_pool(name="sbE", bufs=1) as sbe, \
         tc.tile_pool(name="sbE2", bufs=2) as se2, \
         tc.tile_pool(name="psE", bufs=2, space="PSUM") as pse:
        iot = sbe.tile([128, 128], F16, tag="iotaE", name="iote")
        nc.sync.dma_start(iot[:], IOTA[:])
        off = 0
        for d in range(nch):
            T = t_chunks[d]
            if T == 0:
                continue
            i1 = se2.tile([128, TMAX * 8], I16, tag="i1e", name="i1e")
            nc.sync.dma_start(i1[:, 0:T * 8], IDX1[:, off * 8:(off + T) * 8])
            idd = se2.tile([128, TMAX * 8], I16, tag="idde", name="idde")
            nc.sync.dma_start(idd[:, 0:T * 8], IDXD[:, off * 8:(off + T) * 8])
            dlc = se2.tile([128, TMAX], F16, tag="dlce", name="dlce")
            nc.sync.dma_start(dlc[:, 0:T], DSTLOC[:, off:off + T])

            g2 = se2.tile([128, TMAX * ROW2], F16, tag="g2", name="g2")
            coff = 0
            for s in range(NSCH):
                cl = int(shapes2[d, s])
                if cl == 0:
                    continue
                nc.gpsimd.dma_gather(
                    out_ap=g2[:, coff:(coff + (cl // 128) * ROW2)]
                    .rearrange("p (t e) -> p t e", e=ROW2),
                    in_ap=tab2[s * SCHW:(s + 1) * SCHW, :],
                    idxs_ap=i1[:, coff // 16:(coff + cl) // 16],
                    num_idxs=cl, num_idxs_reg=cl, elem_size=ROW2, single_packet=False)
                coff += cl
            ga2 = se2.tile([128, TMAX * ROWA], F16, tag="ga2", name="ga2")
            nedge = T * 128
            nc.gpsimd.dma_gather(
                out_ap=ga2[:, 0:T * ROWA].rearrange("p (t e) -> p t e", e=ROWA),
                in_ap=adr2[:],
                idxs_ap=idd[:, 0:nedge // 16],
                num_idxs=nedge, num_idxs_reg=nedge, elem_size=ROWA, single_packet=False)

            g23 = g2[:, 0:T * ROW2].rearrange("p (t e) -> p t e", e=ROW2)
            ga23 = ga2[:, 0:T * ROWA].rearrange("p (t e) -> p t e", e=ROWA)

            ind = se2.tile([128, TMAX * 128], F16, tag="inde", name="inde")
            ind3 = ind[:, 0:T * 128].rearrange("p (t s) -> p t s", s=128)
            nc.vector.tensor_tensor(
                out=ind3,
                in0=iot[:].rearrange("p (t s) -> p t s", t=1)
                .to_broadcast([128, T, 128]),
                in1=dlc[:, 0:T].rearrange("p (t s) -> p t s", s=1)
                .to_broadcast([128, T, 128]),
                op=eq)

            at2 = se2.tile([128, TMAX], F16, tag="at2", name="at2")
            at23 = at2[:, 0:T].rearrange("p (t h) -> p t h", h=1)
            nc.vector.tensor_tensor(out=at23,
                                    in0=g23[:, :, NCLASS + 1:NCLASS + 2],
                                    in1=ga23[:, :, 0:1], op=aadd)
            nc.vector.scalar_tensor_tensor(
                out=at23, in0=at23, scalar=NEG_SLOPE, in1=at23,
                op0=mult, op1=amax)
            w2t = se2.tile([128, TMAX], F16, tag="w2t", name="w2t")
            nc.scalar.activation(out=w2t[:, 0:T], in_=at2[:, 0:T], func=AF.Exp)

            nc.vector.tensor_tensor(
                out=ind3, in0=ind3,
                in1=w2t[:, 0:T].rearrange("p (t s) -> p t s", s=1)
                .to_broadcast([128, T, 128]),
                op=mult)

            ps2 = pse.tile([128, NCLASS + 1], F32, tag="ps2", name="ps2")
            for t in range(T):
                nc.tensor.matmul(ps2[:], lhsT=ind[:, t * 128:(t + 1) * 128],
                                 rhs=g2[:, t * ROW2:t * ROW2 + NCLASS + 1],
                                 start=(t == 0), stop=(t == T - 1))

            rc2 = se2.tile([128, 1], F32, tag="rc2", name="rc2")
            nc.vector.reciprocal(rc2[:], ps2[:, NCLASS:NCLASS + 1])
            lg = se2.tile([128, NCLASS], F32, tag="lg", name="lg")
            nc.vector.tensor_scalar_mul(lg[:], ps2[:, 0:NCLASS], rc2[:])
            mx = se2.tile([128, 1], F32, tag="mx", name="mx")
            nc.vector.tensor_reduce(mx[:], lg[:], axis=AX.X, op=amax)
            xm = se2.tile([128, NCLASS], F32, tag="xm", name="xm")
            nc.vector.tensor_scalar(out=xm[:], in0=lg[:], scalar1=mx[:],
                                    scalar2=None, op0=sub)
            ex = se2.tile([128, NCLASS], F32, tag="ex", name="ex")
            sm = se2.tile([128, 1], F32, tag="sm", name="sm")
            nc.scalar.activation(out=ex[:], in_=xm[:], func=AF.Exp,
                                 accum_out=sm[:])
            ls = se2.tile([128, 1], F32, tag="ls", name="ls")
            nc.scalar.activation(out=ls[:], in_=sm[:], func=AF.Ln)
            fin = se2.tile([128, NCLASS], F32, tag="fin", name="fin")
            nc.vector.tensor_scalar(out=fin[:], in0=xm[:], scalar1=ls[:],
                                    scalar2=None, op0=sub)
            nc.sync.dma_start(OUT[d * 128:(d + 1) * 128, :], fin[:])
            off += T


_CACHE = {}


def kernel(x, edge_index, W1, att_src1, att_dst1, b1, W2, att_src2, att_dst2, b2):
    x = np.asarray(x, dtype=np.float32)
    edge_index = np.asarray(edge_index)
    in_maps, shapes2 = _prep(np.asarray(x), edge_index,
                             np.asarray(W1), np.asarray(att_src1),
                             np.asarray(att_dst1), np.asarray(W2),
                             np.asarray(att_src2), np.asarray(att_dst2))
    key = shapes2.tobytes()
    if key not in _CACHE:
        _CACHE[key] = _build(shapes2)
    nc = _CACHE[key]
    res = run_bass_kernel_spmd(nc, in_maps, core_ids=list(range(NCORES)))
    out = np.concatenate([res.results[k]["out"][:NPC] for k in range(NCORES)], axis=0)
    return out.astype(np.float32)


# revision 21
# speedup vs baseline: 2.0637x; 2.0637x over previous
#!/usr/bin/env python3
"""2-layer GAT on 8 NeuronCores (Bass/Tile).

Sharding: nodes partitioned across 8 cores by dst id (graph parallel).
Per-node features computed locally, per-node gather tables allgathered,
per-edge source rows fetched with dma_gather, segment softmax/aggregation
via indicator matmuls on the tensor engine.
"""
import sys
import numpy as np

sys.path.insert(0, "/opt/pypackages")
sys.path.insert(0, "/opt/trn_rl_repo")

import concourse.bass as bass
import concourse.bacc as bacc
import concourse.tile as tile
import concourse.mybir as mybir
from concourse.bass_utils import run_bass_kernel_spmd

# problem constants
N = 100000
F_IN = 512
NHID = 16
HEADS = 8
NCLASS = 40
E = 1600000
NEG_SLOPE = 0.2

NCORES = 8
NPC = N // NCORES            # 12500 nodes per core
DCH = 128                    # dsts per chunk
NCH = (NPC + DCH - 1) // DCH  # 98 chunks
NPAD = NCH * DCH             # 12544 padded rows per core shard
NSCH = 4
SCHW = (NPAD * NCORES) // NSCH  # 25088 src rows per index window (int16-safe)

ROW1 = 256    # fp16 elems per L1 table row (512B): [h1 128 | asrc1 8 | pad]
ROW2 = 128    # fp16 elems per L2 table row (256B): [h2 40 | one | asrc2 | pad]
ROWA = 128    # fp16 elems per adst-replica row (256B)

F16 = mybir.dt.float16
F32 = mybir.dt.float32
I16 = mybir.dt.int16


def _wrap_block(v):
    """Wrap a 1-D int16 block (len % 16 == 0) into dma_gather idx layout
    [16, L/16], replicated to 128 partitions."""
    w = v.reshape(-1, 16).T
    return np.tile(w, (8, 1))



def _dma_gather_raw(gp, out_ap, in_ap, idxs_ap, num_idxs, elem_size, elem_step):
    """dma_gather allowing elem_size (bytes read per row) that is not a
    multiple of 256B; the table row stride (elem_step) still must be."""
    from concourse.bass import exact_div
    stride_bytes = elem_step * mybir.dt.size(in_ap.dtype)
    stride_bytes_256 = exact_div(stride_bytes, 256)
    _in_ap = gp.lower_ap_dma(in_ap, for_custom_bir_dma=True)
    _idxs_ap = gp.lower_ap(idxs_ap)
    _out_ap = gp.lower_ap(out_ap)
    return gp.add_instruction(
        mybir.InstDMAGatherAnt(
            name=gp.bass.get_next_instruction_name(),
            ins=[*_in_ap, _idxs_ap, gp.lower_val_access(gp.to_reg(num_idxs))],
            outs=[_out_ap],
            transpose=False, num_idxs=num_idxs, elem_size=elem_size,
            stride_bytes_256=stride_bytes_256, gen_mode=0,
            single_packet=False, queue_num=0,
            sbuf_tokens_per_rank=0, sbuf_free_dim_per_rank=0,
            sbuf_free_dim_pad_per_rank=0, sbuf_byte_offset=0))


def _prep(x, edge_index, W1, att_src1, att_dst1, W2, att_src2, att_dst2,
          b1=None, b2=None):
    """Host-side sharding/packing. Returns (in_maps, shapes) where shapes is
    the static cell structure shared by all cores."""
    src = np.concatenate([edge_index[0], np.arange(N, dtype=np.int64)])
    dst = np.concatenate([edge_index[1], np.arange(N, dtype=np.int64)])

    core = dst // NPC
    dl = (dst - core * NPC).astype(np.int64)      # local dst 0..12499
    dch = dl >> 7                                  # dst chunk 0..97
    s_pad = (src // NPC) * NPAD + (src % NPC)      # padded global src row
    sch = s_pad // SCHW
    sloc = (s_pad - sch * SCHW).astype(np.int64)   # 0..25087 (int16 ok)

    cell = ((core * NCH + dch) * NSCH + sch).astype(np.int64)
    order = np.argsort(cell * (SCHW + 1) + sloc, kind="stable")
    cell_s, sloc_s, dl_s = cell[order], sloc[order], dl[order]

    ncells = NCORES * NCH * NSCH
    counts = np.bincount(cell_s, minlength=ncells).reshape(NCORES, NCH * NSCH)
    shapes = (np.ceil(counts.max(axis=0) / 128.0).astype(np.int64) * 128)  # [NCH*NSCH]
    cell_starts = np.concatenate([[0], np.cumsum(shapes)])                 # per-core stream offsets
    t_total = int(cell_starts[-1]) // 128

    # rank of each edge within its cell
    group_start = np.concatenate([[0], np.cumsum(counts.reshape(-1))])
    first_of_cell = group_start[cell_s]
    rank = np.arange(len(cell_s)) - first_of_cell
    # destination position within the owning core's padded stream
    pos = cell_starts[cell_s % (NCH * NSCH)] + rank
    core_s = cell_s // (NCH * NSCH)

    L = t_total * 128
    idx1 = np.zeros((NCORES, L), dtype=np.int16)
    idxd = np.zeros((NCORES, L), dtype=np.int16)
    dstloc = np.full((NCORES, L), 255.0, dtype=np.float16)
    idx1[core_s, pos] = sloc_s.astype(np.int16)
    idxd[core_s, pos] = dl_s.astype(np.int16)
    dstloc[core_s, pos] = (dl_s & 127).astype(np.float16)

    # per-chunk tile counts and cell layout
    shapes2 = shapes.reshape(NCH, NSCH)
    # wrapped idx streams
    IDX1 = np.zeros((NCORES, 128, L // 16), dtype=np.int16)
    IDXD = np.zeros((NCORES, 128, L // 16), dtype=np.int16)
    for k in range(NCORES):
        off = 0
        for d in range(NCH):
            chunk_len = int(shapes2[d].sum())
            if chunk_len:
                blk = idxd[k, off:off + chunk_len]
                IDXD[k][:, off // 16:(off + chunk_len) // 16] = _wrap_block(blk)
            coff = off
            for s in range(NSCH):
                cl = int(shapes2[d, s])
                if cl:
                    blk = idx1[k, coff:coff + cl]
                    IDX1[k][:, coff // 16:(coff + cl) // 16] = _wrap_block(blk)
                coff += cl
            off += chunk_len
    DSTLOC = dstloc.reshape(NCORES, t_total, 128).transpose(0, 2, 1).copy()

    # weights
    asrc1 = att_src1.reshape(HEADS, NHID)
    adst1 = att_dst1.reshape(HEADS, NHID)
    W1r = W1.reshape(F_IN, HEADS, NHID)
    W1as = np.einsum("khc,hc->kh", W1r, asrc1)     # [512, 8]
    W1ad = np.einsum("khc,hc->kh", W1r, adst1)
    W1ext = np.concatenate([W1, W1as, W1ad], axis=1).astype(np.float16)  # [512, 144]
    W2as = W2 @ att_src2.reshape(NCLASS, 1)        # [128, 1]
    W2ad = W2 @ att_dst2.reshape(NCLASS, 1)
    W2ext = np.concatenate([W2, W2as, W2ad], axis=1).astype(np.float16)  # [128, 42]

    iota = np.broadcast_to(np.arange(128, dtype=np.float16), (128, 128)).copy()

    in_maps = []
    for k in range(NCORES):
        xs = x[k * NPC:(k + 1) * NPC]              # [12500, 512]
        xT = np.zeros((F_IN, NPAD), dtype=np.float16)
        xT[:, :NPC] = xs.T
        in_maps.append({
            "xT": xT,
            "W1ext": W1ext,
            "W2ext": W2ext,
            "IDX1": IDX1[k],
            "IDXD": IDXD[k],
            "DSTLOC": DSTLOC[k],
            "iota": iota,
            "B1": (np.zeros((1, 128), np.float32) if b1 is None
                   else np.asarray(b1, np.float32).reshape(1, 128)),
            "B2": (np.zeros((1, NCLASS), np.float32) if b2 is None
                   else np.asarray(b2, np.float32).reshape(1, NCLASS)),
        })
    return in_maps, shapes2


def _build(shapes2, nch=NCH, phases="ABCDE", clevel=9):
    """Build the Bass module given the static cell structure [NCH, NSCH]."""
    from concourse.masks import make_identity

    t_chunks = [int(shapes2[d].sum()) // 128 for d in range(NCH)]
    t_total = sum(t_chunks)
    TMAX = max(t_chunks)

    nc = bacc.Bacc("TRN2", target_bir_lowering=False, debug=False,
                   enable_asserts=False, num_devices=NCORES)

    xT = nc.dram_tensor("xT", [F_IN, NPAD], F16, kind="ExternalInput")
    W1e = nc.dram_tensor("W1ext", [F_IN, 144], F16, kind="ExternalInput")
    W2e = nc.dram_tensor("W2ext", [128, 42], F16, kind="ExternalInput")
    IDX1 = nc.dram_tensor("IDX1", [128, t_total * 8], I16, kind="ExternalInput")
    IDXD = nc.dram_tensor("IDXD", [128, t_total * 8], I16, kind="ExternalInput")
    DSTLOC = nc.dram_tensor("DSTLOC", [128, t_total], F16, kind="ExternalInput")
    IOTA = nc.dram_tensor("iota", [128, 128], F16, kind="ExternalInput")
    B1 = nc.dram_tensor("B1", [1, 128], F32, kind="ExternalInput")
    B2 = nc.dram_tensor("B2", [1, NCLASS], F32, kind="ExternalInput")
    OUT = nc.dram_tensor("out", [NPAD, NCLASS], F32, kind="ExternalOutput")

    tab1_sh = nc.dram_tensor("tab1_sh", [NPAD, ROW1], F16, kind="Internal")
    tab1 = nc.dram_tensor("tab1", [NPAD * NCORES, ROW1], F16, kind="Internal",
                          addr_space="Shared")
    tab2_sh = nc.dram_tensor("tab2_sh", [NPAD, ROW2], F16, kind="Internal")
    tab2 = nc.dram_tensor("tab2", [NPAD * NCORES, ROW2], F16, kind="Internal",
                          addr_space="Shared")
    adr1 = nc.dram_tensor("adr1", [NPAD, ROWA], F16, kind="Internal")
    adr2 = nc.dram_tensor("adr2", [NPAD, ROWA], F16, kind="Internal")

    eq = mybir.AluOpType.is_equal
    mult = mybir.AluOpType.mult
    amax = mybir.AluOpType.max
    aadd = mybir.AluOpType.add
    sub = mybir.AluOpType.subtract
    AF = mybir.ActivationFunctionType
    AX = mybir.AxisListType

    with tile.TileContext(nc) as tc:
        if "A" in phases:
            _phase_a(nc, tc, nch, xT, W1e, tab1_sh, adr1)
        if "B" in phases:
            nc.gpsimd.collective_compute(
                "AllGather", mybir.AluOpType.bypass,
                replica_groups=[list(range(NCORES))],
                ins=[tab1_sh[:]], outs=[tab1[:]])
        if "C" in phases:
            _phase_c(nc, tc, nch, shapes2, t_chunks, TMAX, make_identity,
                     IDX1, IDXD, DSTLOC, IOTA, B1, W2e, tab1, adr1, tab2_sh, adr2,
                     eq, mult, amax, aadd, AF, clevel)
        if "D" in phases:
            nc.gpsimd.collective_compute(
                "AllGather", mybir.AluOpType.bypass,
                replica_groups=[list(range(NCORES))],
                ins=[tab2_sh[:]], outs=[tab2[:]])
        if "E" in phases:
            _phase_e(nc, tc, nch, shapes2, t_chunks, TMAX,
                     IDX1, IDXD, DSTLOC, IOTA, B2, tab2, adr2, OUT,
                     eq, mult, amax, aadd, sub, AF, AX)

    nc.compile()
    return nc


def _phase_a(nc, tc, nch, xT, W1e, tab1_sh, adr1):
    with tc.tile_pool(name="sbA", bufs=1) as sba, \
         tc.tile_pool(name="sbA2", bufs=4) as sba2, \
         tc.tile_pool(name="psA", bufs=4, space="PSUM") as psa:
        xts = [sba.tile([128, NPAD], F16, tag=f"xt{k}", name=f"xt{k}")
               for k in range(4)]
        w1s = [sba.tile([128, 144], F16, tag=f"w1{k}", name=f"w1{k}")
               for k in range(4)]
        for k in range(4):
            nc.sync.dma_start(xts[k][:], xT[k * 128:(k + 1) * 128, :])
            nc.sync.dma_start(w1s[k][:], W1e[k * 128:(k + 1) * 128, :])
        for nt in range(nch):
            ps = psa.tile([128, 144], F32, tag="psA", name="psA")
            for k in range(4):
                nc.tensor.matmul(ps[:], lhsT=xts[k][:, nt * 128:(nt + 1) * 128],
                                 rhs=w1s[k][:], start=(k == 0), stop=(k == 3))
            row = sba2.tile([128, 136], F16, tag="row", name="row")
            nc.vector.tensor_copy(row[:], ps[:, 0:136])
            nc.sync.dma_start(tab1_sh[nt * 128:(nt + 1) * 128, 0:136], row[:])
            t8 = sba2.tile([128, 8], F16, tag="t8", name="t8")
            nc.vector.tensor_copy(t8[:], ps[:, 136:144])
            nc.sync.dma_start(adr1[nt * 128:(nt + 1) * 128, 0:8], t8[:])


def _phase_c(nc, tc, nch, shapes2, t_chunks, TMAX, make_identity,
             IDX1, IDXD, DSTLOC, IOTA, B1, W2e, tab1, adr1, tab2_sh, adr2,
             eq, mult, amax, aadd, AF, clevel=9):
    with tc.tile_pool(name="sbC", bufs=1) as sbc, \
         tc.tile_pool(name="sbC2", bufs=3) as sb2, \
         tc.tile_pool(name="psC", bufs=2, space="PSUM") as psc:
        iot = sbc.tile([128, 128], F16, tag="iota", name="iotc")
        nc.sync.dma_start(iot[:], IOTA[:])
        ident = sbc.tile([128, 128], F16, tag="ident", name="ident")
        make_identity(nc, ident[:])
        w2s = sbc.tile([128, 42], F16, tag="w2s", name="w2s")
        nc.sync.dma_start(w2s[:], W2e[:])
        b1t = sbc.tile([128, 128], F32, tag="b1t", name="b1t")
        nc.sync.dma_start(b1t[:], B1[:].to_broadcast([128, 128]))

        off = 0  # tile offset into the edge stream
        for d in range(nch):
            T = t_chunks[d]
            if T == 0:
                continue
            i1 = sb2.tile([128, TMAX * 8], I16, tag="i1", name="i1")
            nc.sync.dma_start(i1[:, 0:T * 8], IDX1[:, off * 8:(off + T) * 8])
            idd = sb2.tile([128, TMAX * 8], I16, tag="idd", name="idd")
            nc.sync.dma_start(idd[:, 0:T * 8], IDXD[:, off * 8:(off + T) * 8])
            dlc = sb2.tile([128, TMAX], F16, tag="dlc", name="dlc")
            nc.sync.dma_start(dlc[:, 0:T], DSTLOC[:, off:off + T])

            g1 = sb2.tile([128, TMAX * ROW1], F16, tag="g1", name="g1")
            coff = 0
            for s in range(NSCH):
                cl = int(shapes2[d, s])
                if cl == 0:
                    continue
                if clevel >= 1:
                    nc.gpsimd.dma_gather(
                        out_ap=g1[:, coff * 2:(coff * 2 + (cl // 128) * ROW1)]
                        .rearrange("p (t e) -> p t e", e=ROW1),
                        in_ap=tab1[s * SCHW:(s + 1) * SCHW, :],
                        idxs_ap=i1[:, coff // 16:(coff + cl) // 16],
                        num_idxs=cl, num_idxs_reg=cl, elem_size=ROW1, single_packet=False)
                coff += cl
            ga = sb2.tile([128, TMAX * 8], F16, tag="ga", name="ga")
            nedge = T * 128
            _dma_gather_raw(nc.gpsimd,
                            ga[:, 0:T * 8].rearrange("p (t e) -> p t e", e=8),
                            adr1[:], idd[:, 0:nedge // 16], nedge, 8, ROWA)

            if clevel < 2:
                dbg = sb2.tile([128, 128], F16, tag="dbg", name="dbg")
                nc.vector.tensor_copy(dbg[:], ga[:, 0:128] if clevel < 1 else g1[:, 0:128])
                nc.sync.dma_start(tab2_sh[d * 128:(d + 1) * 128, 0:128], dbg[:])
                off += T
                continue
            g13 = g1[:, 0:T * ROW1].rearrange("p (t e) -> p t e", e=ROW1)
            ga3 = ga[:, 0:T * 8].rearrange("p (t e) -> p t e", e=8)

            ind = sb2.tile([128, TMAX * 128], F16, tag="ind", name="ind")
            ind3 = ind[:, 0:T * 128].rearrange("p (t s) -> p t s", s=128)
            nc.vector.tensor_tensor(
                out=ind3,
                in0=iot[:].rearrange("p (t s) -> p t s", t=1)
                .to_broadcast([128, T, 128]),
                in1=dlc[:, 0:T].rearrange("p (t s) -> p t s", s=1)
                .to_broadcast([128, T, 128]),
                op=eq)

            att = sb2.tile([128, TMAX * 8], F16, tag="att", name="att")
            at3 = att[:, 0:T * 8].rearrange("p (t h) -> p t h", h=8)
            nc.vector.tensor_tensor(out=at3, in0=g13[:, :, 128:136],
                                    in1=ga3[:, :, 0:8], op=aadd)
            nc.vector.scalar_tensor_tensor(
                out=at3, in0=at3, scalar=NEG_SLOPE, in1=at3, op0=mult, op1=amax)
            wst = sb2.tile([128, TMAX * 8], F16, tag="wst", name="wst")
            nc.scalar.activation(out=wst[:, 0:T * 8], in_=att[:, 0:T * 8],
                                 func=AF.Exp)

            if clevel < 3:
                dbg = sb2.tile([128, 128], F16, tag="dbg", name="dbg")
                nc.vector.tensor_copy(dbg[:, 0:120], ind[:, 0:120])
                nc.vector.tensor_copy(dbg[:, 120:128], wst[:, 0:8])
                nc.sync.dma_start(tab2_sh[d * 128:(d + 1) * 128, 0:128], dbg[:])
                off += T
                continue
            ust = sb2.tile([128, TMAX * 136], F16, tag="ust", name="ust")
            us3 = ust[:, 0:T * 136].rearrange("p (t e) -> p t e", e=136)
            w3 = wst[:, 0:T * 8].rearrange("p (t h) -> p t h", h=8)
            nc.vector.tensor_tensor(
                out=ust[:, 0:T * 136].rearrange("p (t e) -> p t e", e=136)[:, :, 0:128]
                .rearrange("p t (h c) -> p t h c", c=NHID),
                in0=g1[:, 0:T * ROW1].rearrange("p (t e) -> p t e", e=ROW1)[:, :, 0:128]
                .rearrange("p t (h c) -> p t h c", c=NHID),
                in1=wst[:, 0:T * 8].rearrange("p (t h c) -> p t h c", h=8, c=1)
                .to_broadcast([128, T, 8, NHID]),
                op=mult)
            nc.vector.tensor_copy(us3[:, :, 128:136], w3)

            ps1 = psc.tile([128, 136], F32, tag="ps1", name="ps1")
            for t in range(T):
                nc.tensor.matmul(ps1[:], lhsT=ind[:, t * 128:(t + 1) * 128],
                                 rhs=ust[:, t * 136:(t + 1) * 136],
                                 start=(t == 0), stop=(t == T - 1))

            if clevel < 4:
                dbg = sb2.tile([128, 128], F16, tag="dbg", name="dbg")
                nc.vector.tensor_copy(dbg[:], ps1[:, 0:128])
                nc.sync.dma_start(tab2_sh[d * 128:(d + 1) * 128, 0:128], dbg[:])
                off += T
                continue
            rc = sb2.tile([128, 8], F32, tag="rc", name="rc")
            nc.vector.reciprocal(rc[:], ps1[:, 128:136])
            o1 = sb2.tile([128, 128], F32, tag="o1", name="o1")
            nc.vector.tensor_tensor(
                out=o1[:].rearrange("p (h c) -> p h c", c=NHID),
                in0=ps1[:, 0:128].rearrange("p (h c) -> p h c", c=NHID),
                in1=rc[:].rearrange("p (h c) -> p h c", c=1)
                .to_broadcast([128, 8, NHID]),
                op=mult)
            nc.vector.tensor_tensor(out=o1[:], in0=o1[:], in1=b1t[:], op=aadd)
            # elu = max(x,0) + (exp(min(x,0)) - 1)
            t1 = sb2.tile([128, 128], F32, tag="t1", name="t1")
            nc.vector.tensor_scalar_min(t1[:], o1[:], 0.0)
            t2 = sb2.tile([128, 128], F32, tag="t2", name="t2")
            nc.scalar.activation(out=t2[:], in_=t1[:], func=AF.Exp)
            nc.vector.tensor_scalar_add(t2[:], t2[:], -1.0)
            nc.vector.tensor_scalar_max(t1[:], o1[:], 0.0)
            elu = sb2.tile([128, 128], F16, tag="elu", name="elu")
            nc.vector.tensor_tensor(out=elu[:], in0=t1[:], in1=t2[:], op=aadd)

            if clevel < 5:
                nc.sync.dma_start(tab2_sh[d * 128:(d + 1) * 128, 0:128], elu[:])
                off += T
                continue
            psT = psc.tile([128, 128], F16, tag="psT", name="psT")
            nc.tensor.transpose(psT[:], elu[:], ident[:])
            eluT = sb2.tile([128, 128], F16, tag="eluT", name="eluT")
            nc.vector.tensor_copy(eluT[:], psT[:])
            ps2a = psc.tile([128, 42], F32, tag="ps2a", name="ps2a")
            nc.tensor.matmul(ps2a[:], lhsT=eluT[:], rhs=w2s[:],
                             start=True, stop=True)

            h2r = sb2.tile([128, ROW2], F16, tag="h2r", name="h2r")
            nc.vector.tensor_copy(h2r[:, 0:NCLASS], ps2a[:, 0:NCLASS])
            nc.vector.memset(h2r[:, NCLASS:NCLASS + 1], 1.0)
            nc.vector.tensor_copy(h2r[:, NCLASS + 1:NCLASS + 2],
                                  ps2a[:, NCLASS:NCLASS + 1])
            nc.sync.dma_start(tab2_sh[d * 128:(d + 1) * 128, 0:NCLASS + 2],
                              h2r[:, 0:NCLASS + 2])
            a2c = sb2.tile([128, 8], F16, tag="a2c", name="a2c")
            nc.vector.tensor_copy(
                a2c[:].rearrange("p (r h) -> p r h", h=1),
                ps2a[:, 41:42].rearrange("p (r h) -> p r h", r=1)
                .to_broadcast([128, 8, 1]))
            nc.sync.dma_start(adr2[d * 128:(d + 1) * 128, 0:8], a2c[:])
            off += T


def _phase_e(nc, tc, nch, shapes2, t_chunks, TMAX,
             IDX1, IDXD, DSTLOC, IOTA, B2, tab2, adr2, OUT,
             eq, mult, amax, aadd, sub, AF, AX):
    with tc.tile_pool(name="sbE", bufs=1) as sbe, \
         tc.tile_pool(name="sbE2", bufs=3) as se2, \
         tc.tile_pool(name="psE", bufs=4, space="PSUM") as pse:
        iot = sbe.tile([128, 128], F16, tag="iotaE", name="iote")
        nc.sync.dma_start(iot[:], IOTA[:])
        b2t = sbe.tile([128, NCLASS], F32, tag="b2t", name="b2t")
        nc.sync.dma_start(b2t[:], B2[:].to_broadcast([128, NCLASS]))
        off = 0
        for d in range(nch):
            T = t_chunks[d]
            if T == 0:
                continue
            i1 = se2.tile([128, TMAX * 8], I16, tag="i1e", name="i1e")
            nc.sync.dma_start(i1[:, 0:T * 8], IDX1[:, off * 8:(off + T) * 8])
            idd = se2.tile([128, TMAX * 8], I16, tag="idde", name="idde")
            nc.sync.dma_start(idd[:, 0:T * 8], IDXD[:, off * 8:(off + T) * 8])
            dlc = se2.tile([128, TMAX], F16, tag="dlce", name="dlce")
            nc.sync.dma_start(dlc[:, 0:T], DSTLOC[:, off:off + T])

            g2 = se2.tile([128, TMAX * 42], F16, tag="g2", name="g2")
            coff = 0
            for s in range(NSCH):
                cl = int(shapes2[d, s])
                if cl == 0:
                    continue
                _dma_gather_raw(nc.gpsimd,
                                g2[:, (coff // 128) * 42:((coff + cl) // 128) * 42]
                                .rearrange("p (t e) -> p t e", e=42),
                                tab2[s * SCHW:(s + 1) * SCHW, :],
                                i1[:, coff // 16:(coff + cl) // 16], cl, 42, ROW2)
                coff += cl
            ga2 = se2.tile([128, TMAX * 8], F16, tag="ga2", name="ga2")
            nedge = T * 128
            _dma_gather_raw(nc.gpsimd,
                            ga2[:, 0:T * 8].rearrange("p (t e) -> p t e", e=8),
                            adr2[:], idd[:, 0:nedge // 16], nedge, 8, ROWA)

            g23 = g2[:, 0:T * 42].rearrange("p (t e) -> p t e", e=42)
            ga23 = ga2[:, 0:T * 8].rearrange("p (t e) -> p t e", e=8)

            ind = se2.tile([128, TMAX * 128], F16, tag="inde", name="inde")
            ind3 = ind[:, 0:T * 128].rearrange("p (t s) -> p t s", s=128)
            nc.vector.tensor_tensor(
                out=ind3,
                in0=iot[:].rearrange("p (t s) -> p t s", t=1)
                .to_broadcast([128, T, 128]),
                in1=dlc[:, 0:T].rearrange("p (t s) -> p t s", s=1)
                .to_broadcast([128, T, 128]),
                op=eq)

            at2 = se2.tile([128, TMAX], F16, tag="at2", name="at2")
            at23 = at2[:, 0:T].rearrange("p (t h) -> p t h", h=1)
            nc.vector.tensor_tensor(out=at23,
                                    in0=g23[:, :, NCLASS + 1:NCLASS + 2],
                                    in1=ga23[:, :, 0:1], op=aadd)
            nc.vector.scalar_tensor_tensor(
                out=at23, in0=at23, scalar=NEG_SLOPE, in1=at23,
                op0=mult, op1=amax)
            w2t = se2.tile([128, TMAX], F16, tag="w2t", name="w2t")
            nc.scalar.activation(out=w2t[:, 0:T], in_=at2[:, 0:T], func=AF.Exp)

            gw = se2.tile([128, TMAX * 42], F16, tag="gw", name="gw")
            nc.vector.tensor_tensor(
                out=gw[:, 0:T * 42].rearrange("p (t e) -> p t e", e=42),
                in0=g23,
                in1=w2t[:, 0:T].rearrange("p (t s) -> p t s", s=1)
                .to_broadcast([128, T, 42]),
                op=mult)

            ps2 = pse.tile([128, NCLASS + 1], F32, tag="ps2", name="ps2")
            for t in range(T):
                nc.tensor.matmul(ps2[:], lhsT=ind[:, t * 128:(t + 1) * 128],
                                 rhs=gw[:, t * 42:t * 42 + NCLASS + 1],
                                 start=(t == 0), stop=(t == T - 1))

            rc2 = se2.tile([128, 1], F32, tag="rc2", name="rc2")
            nc.vector.reciprocal(rc2[:], ps2[:, NCLASS:NCLASS + 1])
            lg = se2.tile([128, NCLASS], F32, tag="lg", name="lg")
            nc.vector.scalar_tensor_tensor(out=lg[:], in0=ps2[:, 0:NCLASS],
                                           scalar=rc2[:], in1=b2t[:],
                                           op0=mult, op1=aadd)
            ex = se2.tile([128, NCLASS], F32, tag="ex", name="ex")
            sm = se2.tile([128, 1], F32, tag="sm", name="sm")
            nc.scalar.activation(out=ex[:], in_=lg[:], func=AF.Exp,
                                 accum_out=sm[:])
            ls = se2.tile([128, 1], F32, tag="ls", name="ls")
            nc.scalar.activation(out=ls[:], in_=sm[:], func=AF.Ln)
            fin = se2.tile([128, NCLASS], F32, tag="fin", name="fin")
            nc.vector.tensor_scalar(out=fin[:], in0=lg[:], scalar1=ls[:],
                                    scalar2=None, op0=sub)
            nc.sync.dma_start(OUT[d * 128:(d + 1) * 128, :], fin[:])
            off += T


_CACHE = {}


def kernel(x, edge_index, W1, att_src1, att_dst1, b1, W2, att_src2, att_dst2, b2):
    x = np.asarray(x, dtype=np.float32)
    edge_index = np.asarray(edge_index)
    in_maps, shapes2 = _prep(np.asarray(x), edge_index,
                             np.asarray(W1), np.asarray(att_src1),
                             np.asarray(att_dst1), np.asarray(W2),
                             np.asarray(att_src2), np.asarray(att_dst2),
                             b1=b1, b2=b2)
    key = shapes2.tobytes()
    if key not in _CACHE:
        _CACHE[key] = _build(shapes2)
    nc = _CACHE[key]
    res = run_bass_kernel_spmd(nc, in_maps, core_ids=list(range(NCORES)))
    out = np.concatenate([res.results[k]["out"][:NPC] for k in range(NCORES)], axis=0)
    return out.astype(np.float32)


# revision 22
# speedup vs baseline: 2.2811x; 1.1053x over previous
#!/usr/bin/env python3
"""2-layer GAT on 8 NeuronCores (Bass/Tile).

Sharding: nodes partitioned across 8 cores by dst id (graph parallel).
Per-node features computed locally, per-node gather tables allgathered,
per-edge source rows fetched with dma_gather, segment softmax/aggregation
via indicator matmuls on the tensor engine.
"""
import sys
import numpy as np

sys.path.insert(0, "/opt/pypackages")
sys.path.insert(0, "/opt/trn_rl_repo")

import concourse.bass as bass
import concourse.bacc as bacc
import concourse.tile as tile
import concourse.mybir as mybir
from concourse.bass_utils import run_bass_kernel_spmd

# problem constants
N = 100000
F_IN = 512
NHID = 16
HEADS = 8
NCLASS = 40
E = 1600000
NEG_SLOPE = 0.2

NCORES = 8
NPC = N // NCORES            # 12500 nodes per core
DCH = 128                    # dsts per chunk
NCH = (NPC + DCH - 1) // DCH  # 98 chunks
NPAD = NCH * DCH             # 12544 padded rows per core shard
NSCH = 4
SCHW = (NPAD * NCORES) // NSCH  # 25088 src rows per index window (int16-safe)

ROW1 = 256    # fp16 elems per L1 table row (512B): [h1 128 | asrc1 8 | pad]
ROW2 = 128    # fp16 elems per L2 table row (256B): [h2 40 | one | asrc2 | pad]
ROWA = 128    # fp16 elems per adst-replica row (256B)

F16 = mybir.dt.float16
F32 = mybir.dt.float32
I16 = mybir.dt.int16


def _wrap_block(v):
    """Wrap a 1-D int16 block (len % 16 == 0) into dma_gather idx layout
    [16, L/16], replicated to 128 partitions."""
    w = v.reshape(-1, 16).T
    return np.tile(w, (8, 1))



def _dma_gather_raw(gp, out_ap, in_ap, idxs_ap, num_idxs, elem_size, elem_step,
                    queue_num=0):
    """dma_gather allowing elem_size (bytes read per row) that is not a
    multiple of 256B; the table row stride (elem_step) still must be."""
    from concourse.bass import exact_div
    stride_bytes = elem_step * mybir.dt.size(in_ap.dtype)
    stride_bytes_256 = exact_div(stride_bytes, 256)
    _in_ap = gp.lower_ap_dma(in_ap, for_custom_bir_dma=True)
    _idxs_ap = gp.lower_ap(idxs_ap)
    _out_ap = gp.lower_ap(out_ap)
    return gp.add_instruction(
        mybir.InstDMAGatherAnt(
            name=gp.bass.get_next_instruction_name(),
            ins=[*_in_ap, _idxs_ap, gp.lower_val_access(gp.to_reg(num_idxs))],
            outs=[_out_ap],
            transpose=False, num_idxs=num_idxs, elem_size=elem_size,
            stride_bytes_256=stride_bytes_256, gen_mode=0,
            single_packet=False, queue_num=queue_num,
            sbuf_tokens_per_rank=0, sbuf_free_dim_per_rank=0,
            sbuf_free_dim_pad_per_rank=0, sbuf_byte_offset=0))


def _prep(x, edge_index, W1, att_src1, att_dst1, W2, att_src2, att_dst2,
          b1=None, b2=None):
    """Host-side sharding/packing. Returns (in_maps, shapes) where shapes is
    the static cell structure shared by all cores."""
    src = np.concatenate([edge_index[0], np.arange(N, dtype=np.int64)])
    dst = np.concatenate([edge_index[1], np.arange(N, dtype=np.int64)])

    core = dst // NPC
    dl = (dst - core * NPC).astype(np.int64)      # local dst 0..12499
    dch = dl >> 7                                  # dst chunk 0..97
    s_pad = (src // NPC) * NPAD + (src % NPC)      # padded global src row
    sch = s_pad // SCHW
    sloc = (s_pad - sch * SCHW).astype(np.int64)   # 0..25087 (int16 ok)

    cell = ((core * NCH + dch) * NSCH + sch).astype(np.int64)
    order = np.argsort(cell * (SCHW + 1) + sloc, kind="stable")
    cell_s, sloc_s, dl_s = cell[order], sloc[order], dl[order]

    ncells = NCORES * NCH * NSCH
    counts = np.bincount(cell_s, minlength=ncells).reshape(NCORES, NCH * NSCH)
    shapes = (np.ceil(counts.max(axis=0) / 128.0).astype(np.int64) * 128)  # [NCH*NSCH]
    cell_starts = np.concatenate([[0], np.cumsum(shapes)])                 # per-core stream offsets
    t_total = int(cell_starts[-1]) // 128

    # rank of each edge within its cell
    group_start = np.concatenate([[0], np.cumsum(counts.reshape(-1))])
    first_of_cell = group_start[cell_s]
    rank = np.arange(len(cell_s)) - first_of_cell
    # destination position within the owning core's padded stream
    pos = cell_starts[cell_s % (NCH * NSCH)] + rank
    core_s = cell_s // (NCH * NSCH)

    L = t_total * 128
    idx1 = np.zeros((NCORES, L), dtype=np.int16)
    idxd = np.zeros((NCORES, L), dtype=np.int16)
    dstloc = np.full((NCORES, L), 255.0, dtype=np.float16)
    idx1[core_s, pos] = sloc_s.astype(np.int16)
    idxd[core_s, pos] = dl_s.astype(np.int16)
    dstloc[core_s, pos] = (dl_s & 127).astype(np.float16)

    # per-chunk tile counts and cell layout
    shapes2 = shapes.reshape(NCH, NSCH)
    # wrapped idx streams
    IDX1 = np.zeros((NCORES, 128, L // 16), dtype=np.int16)
    IDXD = np.zeros((NCORES, 128, L // 16), dtype=np.int16)
    for k in range(NCORES):
        off = 0
        for d in range(NCH):
            chunk_len = int(shapes2[d].sum())
            if chunk_len:
                blk = idxd[k, off:off + chunk_len]
                IDXD[k][:, off // 16:(off + chunk_len) // 16] = _wrap_block(blk)
            coff = off
            for s in range(NSCH):
                cl = int(shapes2[d, s])
                if cl:
                    blk = idx1[k, coff:coff + cl]
                    IDX1[k][:, coff // 16:(coff + cl) // 16] = _wrap_block(blk)
                coff += cl
            off += chunk_len
    DSTLOC = dstloc.reshape(NCORES, t_total, 128).transpose(0, 2, 1).copy()

    # weights
    asrc1 = att_src1.reshape(HEADS, NHID)
    adst1 = att_dst1.reshape(HEADS, NHID)
    W1r = W1.reshape(F_IN, HEADS, NHID)
    W1as = np.einsum("khc,hc->kh", W1r, asrc1)     # [512, 8]
    W1ad = np.einsum("khc,hc->kh", W1r, adst1)
    W1ext = np.concatenate([W1, W1as, W1ad], axis=1).astype(np.float16)  # [512, 144]
    W2as = W2 @ att_src2.reshape(NCLASS, 1)        # [128, 1]
    W2ad = W2 @ att_dst2.reshape(NCLASS, 1)
    W2ext = np.concatenate([W2, W2as, W2ad], axis=1).astype(np.float16)  # [128, 42]

    iota = np.broadcast_to(np.arange(128, dtype=np.float16), (128, 128)).copy()

    in_maps = []
    for k in range(NCORES):
        xs = x[k * NPC:(k + 1) * NPC]              # [12500, 512]
        xT = np.zeros((F_IN, NPAD), dtype=np.float16)
        xT[:, :NPC] = xs.T
        in_maps.append({
            "xT": xT,
            "W1ext": W1ext,
            "W2ext": W2ext,
            "IDX1": IDX1[k],
            "IDXD": IDXD[k],
            "DSTLOC": DSTLOC[k],
            "iota": iota,
            "B1": (np.zeros((1, 128), np.float32) if b1 is None
                   else np.asarray(b1, np.float32).reshape(1, 128)),
            "B2": (np.zeros((1, NCLASS), np.float32) if b2 is None
                   else np.asarray(b2, np.float32).reshape(1, NCLASS)),
        })
    return in_maps, shapes2


def _build(shapes2, nch=NCH, phases="ABCDE", clevel=9):
    """Build the Bass module given the static cell structure [NCH, NSCH]."""
    from concourse.masks import make_identity

    t_chunks = [int(shapes2[d].sum()) // 128 for d in range(NCH)]
    t_total = sum(t_chunks)
    TMAX = max(t_chunks)

    nc = bacc.Bacc("TRN2", target_bir_lowering=False, debug=False,
                   enable_asserts=False, num_devices=NCORES,
                   num_swdge_queues=4)

    xT = nc.dram_tensor("xT", [F_IN, NPAD], F16, kind="ExternalInput")
    W1e = nc.dram_tensor("W1ext", [F_IN, 144], F16, kind="ExternalInput")
    W2e = nc.dram_tensor("W2ext", [128, 42], F16, kind="ExternalInput")
    IDX1 = nc.dram_tensor("IDX1", [128, t_total * 8], I16, kind="ExternalInput")
    IDXD = nc.dram_tensor("IDXD", [128, t_total * 8], I16, kind="ExternalInput")
    DSTLOC = nc.dram_tensor("DSTLOC", [128, t_total], F16, kind="ExternalInput")
    IOTA = nc.dram_tensor("iota", [128, 128], F16, kind="ExternalInput")
    B1 = nc.dram_tensor("B1", [1, 128], F32, kind="ExternalInput")
    B2 = nc.dram_tensor("B2", [1, NCLASS], F32, kind="ExternalInput")
    OUT = nc.dram_tensor("out", [NPAD, NCLASS], F32, kind="ExternalOutput")

    tab1_sh = nc.dram_tensor("tab1_sh", [NPAD, ROW1], F16, kind="Internal")
    tab1 = nc.dram_tensor("tab1", [NPAD * NCORES, ROW1], F16, kind="Internal",
                          addr_space="Shared")
    tab2_sh = nc.dram_tensor("tab2_sh", [NPAD, ROW2], F16, kind="Internal")
    tab2 = nc.dram_tensor("tab2", [NPAD * NCORES, ROW2], F16, kind="Internal",
                          addr_space="Shared")
    adr1 = nc.dram_tensor("adr1", [NPAD, ROWA], F16, kind="Internal")
    adr2 = nc.dram_tensor("adr2", [NPAD, ROWA], F16, kind="Internal")

    eq = mybir.AluOpType.is_equal
    mult = mybir.AluOpType.mult
    amax = mybir.AluOpType.max
    aadd = mybir.AluOpType.add
    sub = mybir.AluOpType.subtract
    AF = mybir.ActivationFunctionType
    AX = mybir.AxisListType

    with tile.TileContext(nc) as tc:
        if "A" in phases:
            _phase_a(nc, tc, nch, xT, W1e, tab1_sh, adr1)
        if "B" in phases:
            nc.gpsimd.collective_compute(
                "AllGather", mybir.AluOpType.bypass,
                replica_groups=[list(range(NCORES))],
                ins=[tab1_sh[:]], outs=[tab1[:]])
        if "C" in phases:
            _phase_c(nc, tc, nch, shapes2, t_chunks, TMAX, make_identity,
                     IDX1, IDXD, DSTLOC, IOTA, B1, W2e, tab1, adr1, tab2_sh, adr2,
                     eq, mult, amax, aadd, AF, clevel)
        if "D" in phases:
            nc.gpsimd.collective_compute(
                "AllGather", mybir.AluOpType.bypass,
                replica_groups=[list(range(NCORES))],
                ins=[tab2_sh[:]], outs=[tab2[:]])
        if "E" in phases:
            _phase_e(nc, tc, nch, shapes2, t_chunks, TMAX,
                     IDX1, IDXD, DSTLOC, IOTA, B2, tab2, adr2, OUT,
                     eq, mult, amax, aadd, sub, AF, AX)

    nc.compile()
    return nc


def _phase_a(nc, tc, nch, xT, W1e, tab1_sh, adr1):
    with tc.tile_pool(name="sbA", bufs=1) as sba, \
         tc.tile_pool(name="sbA2", bufs=4) as sba2, \
         tc.tile_pool(name="psA", bufs=4, space="PSUM") as psa:
        xts = [sba.tile([128, NPAD], F16, tag=f"xt{k}", name=f"xt{k}")
               for k in range(4)]
        w1s = [sba.tile([128, 144], F16, tag=f"w1{k}", name=f"w1{k}")
               for k in range(4)]
        for k in range(4):
            nc.sync.dma_start(xts[k][:], xT[k * 128:(k + 1) * 128, :])
            nc.sync.dma_start(w1s[k][:], W1e[k * 128:(k + 1) * 128, :])
        for nt in range(nch):
            ps = psa.tile([128, 144], F32, tag="psA", name="psA")
            for k in range(4):
                nc.tensor.matmul(ps[:], lhsT=xts[k][:, nt * 128:(nt + 1) * 128],
                                 rhs=w1s[k][:], start=(k == 0), stop=(k == 3))
            row = sba2.tile([128, 136], F16, tag="row", name="row")
            nc.vector.tensor_copy(row[:], ps[:, 0:136])
            nc.sync.dma_start(tab1_sh[nt * 128:(nt + 1) * 128, 0:136], row[:])
            t8 = sba2.tile([128, 8], F16, tag="t8", name="t8")
            nc.vector.tensor_copy(t8[:], ps[:, 136:144])
            nc.sync.dma_start(adr1[nt * 128:(nt + 1) * 128, 0:8], t8[:])


def _phase_c(nc, tc, nch, shapes2, t_chunks, TMAX, make_identity,
             IDX1, IDXD, DSTLOC, IOTA, B1, W2e, tab1, adr1, tab2_sh, adr2,
             eq, mult, amax, aadd, AF, clevel=9):
    with tc.tile_pool(name="sbC", bufs=1) as sbc, \
         tc.tile_pool(name="sbC2", bufs=3) as sb2, \
         tc.tile_pool(name="psC", bufs=2, space="PSUM") as psc:
        iot = sbc.tile([128, 128], F16, tag="iota", name="iotc")
        nc.sync.dma_start(iot[:], IOTA[:])
        ident = sbc.tile([128, 128], F16, tag="ident", name="ident")
        make_identity(nc, ident[:])
        w2s = sbc.tile([128, 42], F16, tag="w2s", name="w2s")
        nc.sync.dma_start(w2s[:], W2e[:])
        b1t = sbc.tile([128, 128], F32, tag="b1t", name="b1t")
        nc.sync.dma_start(b1t[:], B1[:].to_broadcast([128, 128]))

        off = 0  # tile offset into the edge stream
        for d in range(nch):
            T = t_chunks[d]
            if T == 0:
                continue
            i1 = sb2.tile([128, TMAX * 8], I16, tag="i1", name="i1")
            nc.sync.dma_start(i1[:, 0:T * 8], IDX1[:, off * 8:(off + T) * 8])
            idd = sb2.tile([128, TMAX * 8], I16, tag="idd", name="idd")
            nc.sync.dma_start(idd[:, 0:T * 8], IDXD[:, off * 8:(off + T) * 8])
            dlc = sb2.tile([128, TMAX], F16, tag="dlc", name="dlc")
            nc.sync.dma_start(dlc[:, 0:T], DSTLOC[:, off:off + T])

            g1 = sb2.tile([128, TMAX * ROW1], F16, tag="g1", name="g1")
            coff = 0
            for s in range(NSCH):
                cl = int(shapes2[d, s])
                if cl == 0:
                    continue
                if clevel >= 1:
                    nc.gpsimd.dma_gather(
                        out_ap=g1[:, coff * 2:(coff * 2 + (cl // 128) * ROW1)]
                        .rearrange("p (t e) -> p t e", e=ROW1),
                        in_ap=tab1[s * SCHW:(s + 1) * SCHW, :],
                        idxs_ap=i1[:, coff // 16:(coff + cl) // 16],
                        num_idxs=cl, num_idxs_reg=cl, elem_size=ROW1, single_packet=False)
                coff += cl
            ga = sb2.tile([128, TMAX * 8], F16, tag="ga", name="ga")
            nedge = T * 128
            _dma_gather_raw(nc.gpsimd,
                            ga[:, 0:T * 8].rearrange("p (t e) -> p t e", e=8),
                            adr1[:], idd[:, 0:nedge // 16], nedge, 8, ROWA,
                            queue_num=d % 4)

            if clevel < 2:
                dbg = sb2.tile([128, 128], F16, tag="dbg", name="dbg")
                nc.vector.tensor_copy(dbg[:], ga[:, 0:128] if clevel < 1 else g1[:, 0:128])
                nc.sync.dma_start(tab2_sh[d * 128:(d + 1) * 128, 0:128], dbg[:])
                off += T
                continue
            g13 = g1[:, 0:T * ROW1].rearrange("p (t e) -> p t e", e=ROW1)
            ga3 = ga[:, 0:T * 8].rearrange("p (t e) -> p t e", e=8)

            ind = sb2.tile([128, TMAX * 128], F16, tag="ind", name="ind")
            ind3 = ind[:, 0:T * 128].rearrange("p (t s) -> p t s", s=128)
            nc.vector.tensor_tensor(
                out=ind3,
                in0=iot[:].rearrange("p (t s) -> p t s", t=1)
                .to_broadcast([128, T, 128]),
                in1=dlc[:, 0:T].rearrange("p (t s) -> p t s", s=1)
                .to_broadcast([128, T, 128]),
                op=eq)

            att = sb2.tile([128, TMAX * 8], F16, tag="att", name="att")
            at3 = att[:, 0:T * 8].rearrange("p (t h) -> p t h", h=8)
            nc.vector.tensor_tensor(out=at3, in0=g13[:, :, 128:136],
                                    in1=ga3[:, :, 0:8], op=aadd)
            nc.vector.scalar_tensor_tensor(
                out=at3, in0=at3, scalar=NEG_SLOPE, in1=at3, op0=mult, op1=amax)
            wst = sb2.tile([128, TMAX * 8], F16, tag="wst", name="wst")
            nc.scalar.activation(out=wst[:, 0:T * 8], in_=att[:, 0:T * 8],
                                 func=AF.Exp)

            if clevel < 3:
                dbg = sb2.tile([128, 128], F16, tag="dbg", name="dbg")
                nc.vector.tensor_copy(dbg[:, 0:120], ind[:, 0:120])
                nc.vector.tensor_copy(dbg[:, 120:128], wst[:, 0:8])
                nc.sync.dma_start(tab2_sh[d * 128:(d + 1) * 128, 0:128], dbg[:])
                off += T
                continue
            ust = sb2.tile([128, TMAX * 136], F16, tag="ust", name="ust")
            us3 = ust[:, 0:T * 136].rearrange("p (t e) -> p t e", e=136)
            w3 = wst[:, 0:T * 8].rearrange("p (t h) -> p t h", h=8)
            nc.vector.tensor_tensor(
                out=ust[:, 0:T * 136].rearrange("p (t e) -> p t e", e=136)[:, :, 0:128]
                .rearrange("p t (h c) -> p t h c", c=NHID),
                in0=g1[:, 0:T * ROW1].rearrange("p (t e) -> p t e", e=ROW1)[:, :, 0:128]
                .rearrange("p t (h c) -> p t h c", c=NHID),
                in1=wst[:, 0:T * 8].rearrange("p (t h c) -> p t h c", h=8, c=1)
                .to_broadcast([128, T, 8, NHID]),
                op=mult)
            nc.vector.tensor_copy(us3[:, :, 128:136], w3)

            ps1 = psc.tile([128, 136], F32, tag="ps1", name="ps1")
            for t in range(T):
                nc.tensor.matmul(ps1[:], lhsT=ind[:, t * 128:(t + 1) * 128],
                                 rhs=ust[:, t * 136:(t + 1) * 136],
                                 start=(t == 0), stop=(t == T - 1))

            if clevel < 4:
                dbg = sb2.tile([128, 128], F16, tag="dbg", name="dbg")
                nc.vector.tensor_copy(dbg[:], ps1[:, 0:128])
                nc.sync.dma_start(tab2_sh[d * 128:(d + 1) * 128, 0:128], dbg[:])
                off += T
                continue
            rc = sb2.tile([128, 8], F32, tag="rc", name="rc")
            nc.vector.reciprocal(rc[:], ps1[:, 128:136])
            o1 = sb2.tile([128, 128], F32, tag="o1", name="o1")
            nc.vector.tensor_tensor(
                out=o1[:].rearrange("p (h c) -> p h c", c=NHID),
                in0=ps1[:, 0:128].rearrange("p (h c) -> p h c", c=NHID),
                in1=rc[:].rearrange("p (h c) -> p h c", c=1)
                .to_broadcast([128, 8, NHID]),
                op=mult)
            nc.vector.tensor_tensor(out=o1[:], in0=o1[:], in1=b1t[:], op=aadd)
            # elu = max(x,0) + (exp(min(x,0)) - 1)
            t1 = sb2.tile([128, 128], F32, tag="t1", name="t1")
            nc.vector.tensor_scalar_min(t1[:], o1[:], 0.0)
            t2 = sb2.tile([128, 128], F32, tag="t2", name="t2")
            nc.scalar.activation(out=t2[:], in_=t1[:], func=AF.Exp)
            nc.vector.tensor_scalar_add(t2[:], t2[:], -1.0)
            nc.vector.tensor_scalar_max(t1[:], o1[:], 0.0)
            elu = sb2.tile([128, 128], F16, tag="elu", name="elu")
            nc.vector.tensor_tensor(out=elu[:], in0=t1[:], in1=t2[:], op=aadd)

            if clevel < 5:
                nc.sync.dma_start(tab2_sh[d * 128:(d + 1) * 128, 0:128], elu[:])
                off += T
                continue
            psT = psc.tile([128, 128], F16, tag="psT", name="psT")
            nc.tensor.transpose(psT[:], elu[:], ident[:])
            eluT = sb2.tile([128, 128], F16, tag="eluT", name="eluT")
            nc.vector.tensor_copy(eluT[:], psT[:])
            ps2a = psc.tile([128, 42], F32, tag="ps2a", name="ps2a")
            nc.tensor.matmul(ps2a[:], lhsT=eluT[:], rhs=w2s[:],
                             start=True, stop=True)

            h2r = sb2.tile([128, ROW2], F16, tag="h2r", name="h2r")
            nc.vector.tensor_copy(h2r[:, 0:NCLASS], ps2a[:, 0:NCLASS])
            nc.vector.memset(h2r[:, NCLASS:NCLASS + 1], 1.0)
            nc.vector.tensor_copy(h2r[:, NCLASS + 1:NCLASS + 2],
                                  ps2a[:, NCLASS:NCLASS + 1])
            nc.sync.dma_start(tab2_sh[d * 128:(d + 1) * 128, 0:NCLASS + 2],
                              h2r[:, 0:NCLASS + 2])
            a2c = sb2.tile([128, 8], F16, tag="a2c", name="a2c")
            nc.vector.tensor_copy(
                a2c[:].rearrange("p (r h) -> p r h", h=1),
                ps2a[:, 41:42].rearrange("p (r h) -> p r h", r=1)
                .to_broadcast([128, 8, 1]))
            nc.sync.dma_start(adr2[d * 128:(d + 1) * 128, 0:8], a2c[:])
            off += T


def _phase_e(nc, tc, nch, shapes2, t_chunks, TMAX,
             IDX1, IDXD, DSTLOC, IOTA, B2, tab2, adr2, OUT,
             eq, mult, amax, aadd, sub, AF, AX):
    with tc.tile_pool(name="sbE", bufs=1) as sbe, \
         tc.tile_pool(name="sbE2", bufs=3) as se2, \
         tc.tile_pool(name="psE", bufs=4, space="PSUM") as pse:
        iot = sbe.tile([128, 128], F16, tag="iotaE", name="iote")
        nc.sync.dma_start(iot[:], IOTA[:])
        b2t = sbe.tile([128, NCLASS], F32, tag="b2t", name="b2t")
        nc.sync.dma_start(b2t[:], B2[:].to_broadcast([128, NCLASS]))
        off = 0
        for d in range(nch):
            T = t_chunks[d]
            if T == 0:
                continue
            i1 = se2.tile([128, TMAX * 8], I16, tag="i1e", name="i1e")
            nc.sync.dma_start(i1[:, 0:T * 8], IDX1[:, off * 8:(off + T) * 8])
            idd = se2.tile([128, TMAX * 8], I16, tag="idde", name="idde")
            nc.sync.dma_start(idd[:, 0:T * 8], IDXD[:, off * 8:(off + T) * 8])
            dlc = se2.tile([128, TMAX], F16, tag="dlce", name="dlce")
            nc.sync.dma_start(dlc[:, 0:T], DSTLOC[:, off:off + T])

            g2 = se2.tile([128, TMAX * 42], F16, tag="g2", name="g2")
            coff = 0
            for s in range(NSCH):
                cl = int(shapes2[d, s])
                if cl == 0:
                    continue
                _dma_gather_raw(nc.gpsimd,
                                g2[:, (coff // 128) * 42:((coff + cl) // 128) * 42]
                                .rearrange("p (t e) -> p t e", e=42),
                                tab2[s * SCHW:(s + 1) * SCHW, :],
                                i1[:, coff // 16:(coff + cl) // 16], cl, 42, ROW2,
                                queue_num=s)
                coff += cl
            ga2 = se2.tile([128, TMAX * 8], F16, tag="ga2", name="ga2")
            nedge = T * 128
            _dma_gather_raw(nc.gpsimd,
                            ga2[:, 0:T * 8].rearrange("p (t e) -> p t e", e=8),
                            adr2[:], idd[:, 0:nedge // 16], nedge, 8, ROWA,
                            queue_num=d % 4)

            g23 = g2[:, 0:T * 42].rearrange("p (t e) -> p t e", e=42)
            ga23 = ga2[:, 0:T * 8].rearrange("p (t e) -> p t e", e=8)

            ind = se2.tile([128, TMAX * 128], F16, tag="inde", name="inde")
            ind3 = ind[:, 0:T * 128].rearrange("p (t s) -> p t s", s=128)
            nc.vector.tensor_tensor(
                out=ind3,
                in0=iot[:].rearrange("p (t s) -> p t s", t=1)
                .to_broadcast([128, T, 128]),
                in1=dlc[:, 0:T].rearrange("p (t s) -> p t s", s=1)
                .to_broadcast([128, T, 128]),
                op=eq)

            at2 = se2.tile([128, TMAX], F16, tag="at2", name="at2")
            at23 = at2[:, 0:T].rearrange("p (t h) -> p t h", h=1)
            nc.vector.tensor_tensor(out=at23,
                                    in0=g23[:, :, NCLASS + 1:NCLASS + 2],
                                    in1=ga23[:, :, 0:1], op=aadd)
            nc.vector.scalar_tensor_tensor(
                out=at23, in0=at23, scalar=NEG_SLOPE, in1=at23,
                op0=mult, op1=amax)
            w2t = se2.tile([128, TMAX], F16, tag="w2t", name="w2t")
            nc.scalar.activation(out=w2t[:, 0:T], in_=at2[:, 0:T], func=AF.Exp)

            gw = se2.tile([128, TMAX * 42], F16, tag="gw", name="gw")
            nc.vector.tensor_tensor(
                out=gw[:, 0:T * 42].rearrange("p (t e) -> p t e", e=42),
                in0=g23,
                in1=w2t[:, 0:T].rearrange("p (t s) -> p t s", s=1)
                .to_broadcast([128, T, 42]),
                op=mult)

            ps2 = pse.tile([128, NCLASS + 1], F32, tag="ps2", name="ps2")
            for t in range(T):
                nc.tensor.matmul(ps2[:], lhsT=ind[:, t * 128:(t + 1) * 128],
                                 rhs=gw[:, t * 42:t * 42 + NCLASS + 1],
                                 start=(t == 0), stop=(t == T - 1))

            rc2 = se2.tile([128, 1], F32, tag="rc2", name="rc2")
            nc.vector.reciprocal(rc2[:], ps2[:, NCLASS:NCLASS + 1])
            lg = se2.tile([128, NCLASS], F32, tag="lg", name="lg")
            nc.vector.scalar_tensor_tensor(out=lg[:], in0=ps2[:, 0:NCLASS],
                                           scalar=rc2[:], in1=b2t[:],
                                           op0=mult, op1=aadd)
            ex = se2.tile([128, NCLASS], F32, tag="ex", name="ex")
            sm = se2.tile([128, 1], F32, tag="sm", name="sm")
            nc.scalar.activation(out=ex[:], in_=lg[:], func=AF.Exp,
                                 accum_out=sm[:])
            ls = se2.tile([128, 1], F32, tag="ls", name="ls")
            nc.scalar.activation(out=ls[:], in_=sm[:], func=AF.Ln)
            fin = se2.tile([128, NCLASS], F32, tag="fin", name="fin")
            nc.vector.tensor_scalar(out=fin[:], in0=lg[:], scalar1=ls[:],
                                    scalar2=None, op0=sub)
            nc.sync.dma_start(OUT[d * 128:(d + 1) * 128, :], fin[:])
            off += T


_CACHE = {}


def kernel(x, edge_index, W1, att_src1, att_dst1, b1, W2, att_src2, att_dst2, b2):
    x = np.asarray(x, dtype=np.float32)
    edge_index = np.asarray(edge_index)
    in_maps, shapes2 = _prep(np.asarray(x), edge_index,
                             np.asarray(W1), np.asarray(att_src1),
                             np.asarray(att_dst1), np.asarray(W2),
                             np.asarray(att_src2), np.asarray(att_dst2),
                             b1=b1, b2=b2)
    key = shapes2.tobytes()
    if key not in _CACHE:
        _CACHE[key] = _build(shapes2)
    nc = _CACHE[key]
    res = run_bass_kernel_spmd(nc, in_maps, core_ids=list(range(NCORES)))
    out = np.concatenate([res.results[k]["out"][:NPC] for k in range(NCORES)], axis=0)
    return out.astype(np.float32)
